# revision 52
# baseline (speedup 1.0000x reference)
"""Trainium2 Bass kernel for nn_EpisodicMemory (BitNet projections + memory cross-attention).

kernel(**inputs) takes FULL unsharded numpy inputs, returns FULL output
[8, 4096, 1024] f32. Batch-parallel across 8 NeuronCores; two scalar
AllReduce(max) collectives provide the global BitNet activation scales.

Design ("folded + transposed dataflow"):
  - Fold the key projection into the memory bank:
        C = qWk^T @ mk^T   [E, M]     (computed in phase A, PE otherwise idle)
        sims = s_ck * (qx @ C) + bk@mk^T
    eliminating the qk matmul from the per-tile critical path.
  - Keep x TRANSPOSED on-chip: x is cast to fp16 and PE-transposed into a
    resident SBUF buffer during the phase-A streaming pass (abs-max on the
    way), so qxT = rne(xT/s_x) is elementwise, and
        simsT[m,t]: lhsT=C[e,mcols], rhs=qxT[e,t]
        retT[e,t]:  lhsT=mv[m,ecols] (natural!), rhs=expT[m,t]
        hT = xT + retT/den  (in place, same SBUF buffer)
        out[t,e]:   lhsT=qhT[e,tcols], rhs=qWoT[e,eout]
    so NO h spill, NO qx bounce, NO DRAM transposes at all.
  - Denominator via DVE adds + gpsimd partition_all_reduce (no PE cost).
  - Retrieval matmul runs in fp8e4 DoubleRow (2 k-subtiles per matmul):
    exp values are clipped at 240 (TRN fp8e4 saturates to Inf above) and
    the denominator gets +1e-6 so fully-underflowed rows yield ret=0, not
    NaN. sims/out matmuls stay fp16 (fp8 there exceeds the error budget).
  - Sims/out matmuls fp16 with fp32 PSUM accumulation (BitNet ones exact).
"""

import math
import time

import numpy as np

import concourse.bass as bass
import concourse.tile as tile
from concourse import bacc, bass_isa, mybir
from concourse.bass_utils import run_bass_kernel_spmd

F32 = mybir.dt.float32
F16 = mybir.dt.float16
F8 = mybir.dt.float8e4

N_CORES = 8
MAGIC = 1.5 * (2.0 ** 23)   # fp32 RNE rounding trick
EXP_SHIFT = 8.3             # logit shift: max exp(logit-shift) < 240 (fp8e4 Inf bound)

B, S_FULL, E_DIM, DM_DIM, M_DIM = 8, 4096, 1024, 1024, 2048


def build_nc(S=S_FULL, E=E_DIM, DM=DM_DIM, M=M_DIM, T=512, repeat=1,
             use_collectives=True, with_bias=True, with_bk=False, dbg=False,
             retr_fp8=True, no_den=False, dedup_ldw=False):
    assert S % T == 0 and T % 128 == 0 and E == 1024 and DM == 1024 and M == 2048
    exp_shift = EXP_SHIFT
    NT = S // T          # 8 row tiles
    TS = T // 128        # 4
    NE = E // 128        # 8
    ND = DM // 128       # 8
    NM = M // 128        # 16
    NOH = E // 512       # 2
    NXC = (S // 128) * 2  # 64 x half-chunks

    nc = bacc.Bacc("TRN2", target_bir_lowering=False, debug=False,
                   num_devices=N_CORES)

    x_in = nc.dram_tensor("x", [S, E], F32, kind="ExternalInput").ap()
    mk_in = nc.dram_tensor("memory_keys", [M, DM], F32, kind="ExternalInput").ap()
    mv_in = nc.dram_tensor("memory_values", [M, E], F32, kind="ExternalInput").ap()
    wk_in = nc.dram_tensor("Wk", [DM, E], F32, kind="ExternalInput").ap()
    bk_in = nc.dram_tensor("bk", [DM], F32, kind="ExternalInput").ap()
    wo_in = nc.dram_tensor("Wo", [E, E], F32, kind="ExternalInput").ap()
    bo_in = nc.dram_tensor("bo", [E], F32, kind="ExternalInput").ap()
    out_ext = nc.dram_tensor("out", [S, E], F32, kind="ExternalOutput").ap()
    if dbg:
        dbg_C = nc.dram_tensor("dbg_C", [128, 8 * M], F16, kind="ExternalOutput").ap()
        dbg_qxT = nc.dram_tensor("dbg_qxT", [128, 8 * T], F16, kind="ExternalOutput").ap()
        dbg_expT = nc.dram_tensor("dbg_expT", [128, 16 * T], F8 if retr_fp8 else F16, kind="ExternalOutput").ap()
        dbg_inv = nc.dram_tensor("dbg_inv", [128, T], F32, kind="ExternalOutput").ap()
        dbg_hT = nc.dram_tensor("dbg_hT", [128, 8 * S], F16, kind="ExternalOutput").ap()
        dbg_sc = nc.dram_tensor("dbg_sc", [1, 8], F32, kind="ExternalOutput").ap()

    import contextlib
    with tile.TileContext(nc) as tc:
        loop_cm = tc.For_i(0, repeat, 1) if repeat > 1 else contextlib.nullcontext()
        with loop_cm:
          with (
            tc.tile_pool(name="pp", bufs=1) as pp,
            tc.tile_pool(name="wp", bufs=2) as wp,
            tc.tile_pool(name="psp", bufs=2, space="PSUM") as psp,
            tc.tile_pool(name="dp", bufs=1, space="DRAM") as dp,
          ):
            # ---------------- persistent SBUF ----------------
            EV = F8 if retr_fp8 else F16
            hT = pp.tile([128, NE, S], F16, tag="hT")        # xT then hT, 64KB/p
            C_sb = pp.tile([128, NE, M], F16, tag="C_sb")    # 32KB/p
            mv_sb = pp.tile([128, NM, E], EV, tag="mv_sb")   # 16/32KB/p
            qWk = pp.tile([128, ND, E], F16, tag="qWk")      # 16KB/p (phase A)
            qWo = pp.tile([128, NE, E], F16, tag="qWo")      # 16KB/p
            big2 = pp.tile([128, ND, T], F16, tag="big2")    # mkT chunk, 8KB/p
            if retr_fp8:
                expT = pp.tile([128, NM, T], F8, tag="expT")  # 8KB/p
            else:
                # fp16 fallback (debug only; qWk now hosts the qxT buffers)
                expT = pp.tile([128, NM, T], F16, tag="expT16")

            rk_sb = pp.tile([128, NM], F32, tag="rk_sb")     # rk/32 - 8 per mb
            bk_sb = pp.tile([128, ND], F32, tag="bk_sb")
            bk16 = pp.tile([128, ND], F16, tag="bk16")
            bo_sb = pp.tile([1, E], F32, tag="bo_sb")
            bo_sc = pp.tile([1, E], F16, tag="bo_sc")
            ones_row = pp.tile([1, 128], F16, tag="ones_row")
            ident16 = pp.tile([128, 128], F16, tag="ident16")
            xmax_buf = pp.tile([128, NXC], F32, tag="xmax_buf")
            hmax_buf = pp.tile([128, NT * NE], F32, tag="hmax_buf")
            xmax_red = pp.tile([128, 1], F32, tag="xmax_red")
            hmax_red = pp.tile([128, 1], F32, tag="hmax_red")
            pr_max = pp.tile([128, 1], F32, tag="pr_max")
            pr_add = pp.tile([128, 1], F32, tag="pr_add")

            sc = {}
            for nm in ("gmax_x", "s_x", "inv_sx", "ws_k", "thr_k", "nthr_k",
                       "ws_o", "thr_o", "nthr_o", "sck32", "gmax_h", "s_h",
                       "inv_sh", "sco", "inv_sco", "red1", "red2", "inv_wso",
                       "inv_wsk"):
                sc[nm] = pp.tile([1, 8], F32, name=f"sc_{nm}", tag=f"sc_{nm}")
            bc = {}
            for nm in ("inv_sx", "thr_k", "nthr_k", "thr_o", "nthr_o", "sck32",
                       "inv_sh", "sco", "inv_wso", "inv_wsk"):
                bc[nm] = pp.tile([128, 1], F32, name=f"bc_{nm}", tag=f"bc_{nm}")

            magic_bc = pp.tile([128, 1], F32, tag="magic_bc")
            nc.vector.memset(magic_bc[:], MAGIC)
            nc.vector.memset(ones_row[:], 1.0)
            from concourse.masks import make_identity
            make_identity(nc, ident16[:])

            rg = [list(range(N_CORES))]

            def allreduce_max(src_scalar, dst_scalar):
                if not use_collectives:
                    nc.vector.tensor_copy(dst_scalar[0:1, 0:1], src_scalar[0:1, 0:1])
                    return
                ccin = dp.tile([1, 8], F32, tag="cc_in", bufs=2, name="ccin")
                ccout = dp.tile([1, 8], F32, addr_space="Shared", tag="cc_out",
                                bufs=2, name="ccout")
                nc.sync.dma_start(ccin[:], src_scalar[:])
                nc.gpsimd.collective_compute(
                    "AllReduce", mybir.AluOpType.max, replica_groups=rg,
                    ins=[ccin[:]], outs=[ccout[:]])
                nc.sync.dma_start(dst_scalar[:], ccout[:])

            def part_reduce_scalar(vec128, out_scalar, op):
                red = bass_isa.ReduceOp.max if op == "max" else bass_isa.ReduceOp.add
                dst = pr_max if op == "max" else pr_add
                nc.gpsimd.partition_all_reduce(dst[:], vec128[:], channels=128,
                                               reduce_op=red)
                nc.vector.tensor_copy(out_scalar[0:1, 0:1], dst[0:1, 0:1])

            # ============ PHASE A ============
            # gpsimd ring: bk, Wk (2-pass), mk chunks, [mv after C], Wo in B
            # sync ring: x chunk loads + xT transposes; scalar ring: xf16 stores
            nc.gpsimd.dma_start(bk_sb[:], bk_in.rearrange("(b p) -> p b", p=128))
            nc.vector.tensor_copy(bk16[:], bk_sb[:])
            nc.gpsimd.dma_start(bo_sb[0:1, :],
                                bo_in.rearrange("(a e) -> a e", a=1))

            # ---- Wk pass 1: mean|Wk| (scalar ring; DVE reduces) ----
            wk_acc = wp.tile([128, 16], F32, tag="wk_acc", bufs=1, name="wk_acc")
            for i in range(16):
                wt = wp.tile([128, 512], F32, tag="wt", bufs=2, name="wt")
                nc.scalar.dma_start(
                    wt[:], wk_in[(i // 2) * 128:(i // 2 + 1) * 128,
                                 (i % 2) * 512:(i % 2) * 512 + 512])
                nc.vector.tensor_reduce(wk_acc[:, i:i + 1], wt[:],
                                        axis=mybir.AxisListType.X,
                                        op=mybir.AluOpType.add,
                                        apply_absolute_value=True)
            wk_accr = wp.tile([128, 1], F32, tag="wk_accr", bufs=1, name="wk_accr")
            nc.vector.tensor_reduce(wk_accr[:], wk_acc[:], axis=mybir.AxisListType.X,
                                    op=mybir.AluOpType.add)
            part_reduce_scalar(wk_accr, sc["red2"], "add")
            nc.vector.tensor_scalar(sc["ws_k"][0:1, 0:1], sc["red2"][0:1, 0:1],
                                    1.0 / (DM * E), None, op0=mybir.AluOpType.mult)
            nc.vector.reciprocal(sc["inv_wsk"][0:1, 0:1], sc["ws_k"][0:1, 0:1])
            nc.gpsimd.partition_broadcast(bc["inv_wsk"][:],
                                          sc["inv_wsk"][0:1, 0:1])

            # ---- helpers for the interleaved phase A main loop ----
            def x_chunk(cx):
                # abs-max + cast fp16 + PE-transpose into hT (grouped copies)
                xc16 = wp.tile([128, E], F16, tag="c16", bufs=2, name="xc16")
                for hf in range(2):
                    cf = slice(hf * 512, (hf + 1) * 512)
                    xc32 = wp.tile([128, 512], F32, tag="xc32", bufs=3, name="xc32")
                    nc.sync.dma_start(xc32[:], x_in[cx * 128:(cx + 1) * 128, cf])
                    nc.vector.tensor_reduce(xmax_buf[:, cx * 2 + hf:cx * 2 + hf + 1],
                                            xc32[:],
                                            axis=mybir.AxisListType.X,
                                            op=mybir.AluOpType.max,
                                            apply_absolute_value=True)
                    if hf == 0:
                        nc.scalar.activation(xc16[:, cf], xc32[:],
                                             mybir.ActivationFunctionType.Copy)
                    else:
                        nc.vector.tensor_copy(xc16[:, cf], xc32[:])
                for g in range(2):
                    tpg = psp.tile([128, 512], F16, tag="ps_x", bufs=2, name="x_ps")
                    for j in range(4):
                        eb = g * 4 + j
                        nc.tensor.transpose(tpg[:, j * 128:(j + 1) * 128],
                                            xc16[:, eb * 128:(eb + 1) * 128],
                                            ident16[:])
                    dst = hT[:, g * 4:(g + 1) * 4, cx * 128:(cx + 1) * 128]
                    if g == 0:
                        nc.scalar.activation(dst, tpg[:],
                                             mybir.ActivationFunctionType.Copy)
                    else:
                        nc.vector.tensor_copy(dst, tpg[:])

            def wk_pass2_chunk(i):
                # ternarize = clip(rne(w/ws_k), -1, 1): ACT magic-round +
                # DVE un-bias/clip (no compare ops)
                db, ch = i // 2, i % 2
                wt = wp.tile([128, 512], F32, tag="wt", bufs=2, name="wt2")
                nc.scalar.dma_start(
                    wt[:], wk_in[db * 128:(db + 1) * 128,
                                 ch * 512:(ch + 1) * 512])
                t = wp.tile([128, 512], F32, tag="f32s", bufs=2, name="wk_t")
                nc.scalar.activation(
                    t[:], wt[:], mybir.ActivationFunctionType.Identity,
                    bias=magic_bc[:, 0:1], scale=bc["inv_wsk"][:, 0:1])
                q = wp.tile([128, 512], F16, tag="wk_q", bufs=1, name="wk_q")
                nc.vector.tensor_scalar(q[:], t[:], MAGIC, None,
                                        op0=mybir.AluOpType.subtract)
                nc.vector.tensor_scalar(qWk[:, db, ch * 512:(ch + 1) * 512],
                                        q[:], 1.0, -1.0,
                                        op0=mybir.AluOpType.min,
                                        op1=mybir.AluOpType.max)

            def mk_block(blk):
                mrow = blk * 128
                mbb = blk % 4
                mk16 = wp.tile([128, E], F16, tag="mk16", bufs=2, name="mk16")
                for ch in range(2):
                    mkt = wp.tile([128, 512], F32, tag="mvt", bufs=2, name="mkt")
                    nc.scalar.dma_start(
                        mkt[:], mk_in[mrow:mrow + 128, ch * 512:(ch + 1) * 512])
                    if ch == 0:
                        nc.scalar.activation(mk16[:, ch * 512:(ch + 1) * 512],
                                             mkt[:],
                                             mybir.ActivationFunctionType.Copy)
                    else:
                        nc.vector.tensor_copy(mk16[:, ch * 512:(ch + 1) * 512],
                                              mkt[:])
                for g in range(2):
                    tpg = psp.tile([128, 512], F16, tag="ps_x", bufs=2, name="mk_ps")
                    for j in range(4):
                        db = g * 4 + j
                        nc.tensor.transpose(tpg[:, j * 128:(j + 1) * 128],
                                            mk16[:, db * 128:(db + 1) * 128],
                                            ident16[:])
                    dst = big2[:, g * 4:(g + 1) * 4, mbb * 128:(mbb + 1) * 128]
                    if g == 0:
                        nc.scalar.activation(dst, tpg[:],
                                             mybir.ActivationFunctionType.Copy)
                    else:
                        nc.vector.tensor_copy(dst, tpg[:])

            def c_chunk(mch):
                for eb in range(NE):
                    psc = psp.tile([128, 512], F32, tag="ps_r", bufs=3,
                                   name="c_ps")
                    for db in range(ND):
                        nc.tensor.matmul(
                            psc[:], qWk[:, db, eb * 128:(eb + 1) * 128],
                            big2[:, db, :],
                            start=(db == 0), stop=(db == ND - 1))
                    if eb % 2 == 0:
                        nc.scalar.activation(
                            C_sb[:, eb, mch * 512:(mch + 1) * 512], psc[:],
                            mybir.ActivationFunctionType.Copy)
                    else:
                        nc.vector.tensor_copy(
                            C_sb[:, eb, mch * 512:(mch + 1) * 512], psc[:])
                if with_bk:
                    for mbb in range(4):
                        mb = mch * 4 + mbb
                        psk = psp.tile([128, 8], F32, tag="ps_k", name="rk_ps")
                        for db in range(ND):
                            nc.tensor.matmul(
                                psk[:, 0:1],
                                big2[:, db, mbb * 128:(mbb + 1) * 128],
                                bk16[:, db:db + 1],
                                start=(db == 0), stop=(db == ND - 1))
                        nc.scalar.activation(
                            rk_sb[:, mb:mb + 1], psk[:, 0:1],
                            mybir.ActivationFunctionType.Copy,
                            bias=-exp_shift, scale=1.0 / math.sqrt(DM))

            # ---- interleaved phase A main loop ----
            if not with_bk:
                nc.vector.memset(rk_sb[:], -exp_shift)
            for i in range(S // 128):
                if 1 <= i < 9:
                    wk_pass2_chunk(2 * (i - 1))
                    wk_pass2_chunk(2 * (i - 1) + 1)
                if i in (10, 14, 18, 22):
                    c_chunk((i - 10) // 4)  # before mk_block overwrites big2
                if 6 <= i < 22:
                    mk_block(i - 6)
                x_chunk(i)

            nc.vector.tensor_reduce(xmax_red[:], xmax_buf[:],
                                    axis=mybir.AxisListType.X,
                                    op=mybir.AluOpType.max)
            part_reduce_scalar(xmax_red, sc["red1"], "max")
            allreduce_max(sc["red1"], sc["gmax_x"])
            nc.vector.tensor_scalar(sc["s_x"][0:1, 0:1], sc["gmax_x"][0:1, 0:1],
                                    1.0 / 127.0, None, op0=mybir.AluOpType.mult)
            nc.vector.reciprocal(sc["inv_sx"][0:1, 0:1], sc["s_x"][0:1, 0:1])
            nc.gpsimd.partition_broadcast(bc["inv_sx"][:], sc["inv_sx"][0:1, 0:1])
            # sck32 = ws_k * s_x / sqrt(DM)
            nc.vector.tensor_tensor(sc["sck32"][0:1, 0:1], sc["ws_k"][0:1, 0:1],
                                    sc["s_x"][0:1, 0:1], op=mybir.AluOpType.mult)
            nc.vector.tensor_scalar(sc["sck32"][0:1, 0:1], sc["sck32"][0:1, 0:1],
                                    1.0 / math.sqrt(DM), None,
                                    op0=mybir.AluOpType.mult)
            nc.gpsimd.partition_broadcast(bc["sck32"][:], sc["sck32"][0:1, 0:1])

            # ======================= PHASE B =======================
            # qxT double-buffer lives in qWk's SBUF (dead after C is built):
            # [128, ND, E] f16 -> [128, 2, NE, T] f16
            qxT_bufs = qWk.rearrange("p a (b c) -> p (a b) c", b=2, c=T) \
                          .rearrange("p (z e) c -> p z e c", z=2, e=NE)

            def quantize_qxT_eb(qxT, it, eb):
                # magic-round split across ACT (mul+add) and DVE (sub)
                qt = wp.tile([128, T], F32, tag="qt_s", bufs=2, name="qt")
                nc.scalar.activation(
                    qt[:], hT[:, eb, it * T:(it + 1) * T],
                    mybir.ActivationFunctionType.Identity,
                    bias=magic_bc[:, 0:1], scale=bc["inv_sx"][:, 0:1])
                nc.vector.tensor_scalar(
                    qxT[:, eb, :], qt[:], MAGIC, None,
                    op0=mybir.AluOpType.subtract)

            def quantize_qxT(it):
                qxT = qxT_bufs[:, it % 2]
                for eb in range(NE):
                    quantize_qxT_eb(qxT, it, eb)
                return qxT

            # Wo 2-pass ternarize into qWo, sliced into closures so the work
            # spreads across phase-B tiles 1-5 (DMA on the otherwise-idle
            # gpsimd/scalar rings, DVE/PE crumbs under each tile)
            wo_acc = wp.tile([128, 16], F32, tag="wk_acc", bufs=1, name="wo_acc")

            def wo_p1(i):
                wt = wp.tile([128, 512], F32, tag="mvt", bufs=2, name="wot")
                nc.gpsimd.dma_start(
                    wt[:], wo_in[(i // 2) * 128:(i // 2 + 1) * 128,
                                 (i % 2) * 512:(i % 2) * 512 + 512])
                nc.vector.tensor_reduce(wo_acc[:, i:i + 1], wt[:],
                                        axis=mybir.AxisListType.X,
                                        op=mybir.AluOpType.add,
                                        apply_absolute_value=True)

            def wo_thr():
                wo_accr = wp.tile([128, 1], F32, tag="wk_accr", bufs=1,
                                  name="wo_accr")
                nc.vector.tensor_reduce(wo_accr[:], wo_acc[:],
                                        axis=mybir.AxisListType.X,
                                        op=mybir.AluOpType.add)
                part_reduce_scalar(wo_accr, sc["red2"], "add")
                nc.vector.tensor_scalar(sc["ws_o"][0:1, 0:1], sc["red2"][0:1, 0:1],
                                        1.0 / (E * E), None,
                                        op0=mybir.AluOpType.mult)
                nc.vector.reciprocal(sc["inv_wso"][0:1, 0:1], sc["ws_o"][0:1, 0:1])
                nc.gpsimd.partition_broadcast(bc["inv_wso"][:],
                                              sc["inv_wso"][0:1, 0:1])

            def wo_p2(ob, ch):
                # ternarize = clip(rne(w/ws_o), -1, 1): ACT magic-round,
                # Pool un-bias + clip -- zero DVE/PE cost under phase-B
                # tiles. Raw (untransposed) block parks in its own qWo
                # region; wo_transpose fixes the layout at the B->C gap.
                wt = wp.tile([128, 512], F32, tag="mvt", bufs=2, name="wot2")
                nc.scalar.dma_start(
                    wt[:], wo_in[ob * 128:(ob + 1) * 128,
                                 ch * 512:(ch + 1) * 512])
                t = wp.tile([128, 512], F32, tag="f32s", bufs=2, name="wo_t")
                nc.scalar.activation(
                    t[:], wt[:], mybir.ActivationFunctionType.Identity,
                    bias=magic_bc[:, 0:1], scale=bc["inv_wso"][:, 0:1])
                q = wp.tile([128, 512], F32, tag="f32s", bufs=2, name="wo_q")
                nc.gpsimd.tensor_scalar(q[:], t[:], MAGIC, None,
                                        op0=mybir.AluOpType.subtract)
                nc.gpsimd.tensor_scalar(
                    qWo[:, ch * 4:(ch + 1) * 4, ob * 128:(ob + 1) * 128],
                    q[:], 1.0, -1.0,
                    op0=mybir.AluOpType.min, op1=mybir.AluOpType.max)

            def wo_transpose():
                # in-place per-region transpose into phase C rhs layout
                # [i_p, ib, o]; runs in the B->C hmax/allreduce shadow
                for ob in range(NE):
                    for ch in range(2):
                        reg = qWo[:, ch * 4:(ch + 1) * 4,
                                  ob * 128:(ob + 1) * 128]
                        tpg = psp.tile([128, 512], F16, tag="ps_x", bufs=2,
                                       name="wo_ps")
                        for j in range(4):
                            nc.tensor.transpose(
                                tpg[:, j * 128:(j + 1) * 128],
                                qWo[:, ch * 4 + j, ob * 128:(ob + 1) * 128],
                                ident16[:])
                        if (ob + ch) % 2 == 0:
                            nc.scalar.activation(
                                reg, tpg[:],
                                mybir.ActivationFunctionType.Copy)
                        else:
                            nc.vector.tensor_copy(reg, tpg[:])

            wo_work = (
                [(lambda i=i: wo_p1(i)) for i in range(16)]
                + [wo_thr]
                + [(lambda ob=ob, ch=ch: wo_p2(ob, ch))
                   for ob in range(NE) for ch in range(2)]
            )
            # cumulative slice boundaries per phase-B tile index (tiles 1..6)
            wo_slices = {1: 8, 2: 17, 3: 21, 4: 25, 5: 29, 6: 33}
            assert wo_slices[6] == len(wo_work)

            qxT = quantize_qxT(0)
            if dbg:
                nc.sync.dma_start(dbg_C[:], C_sb.rearrange("p a b -> p (a b)")[:])
                nc.sync.dma_start(dbg_qxT[:], qxT.rearrange("p a b -> p (a b)")[:])
                sc_dump = pp.tile([1, 8], F32, tag="sc_dump")
                nc.vector.memset(sc_dump[:], 0.0)
                nc.vector.tensor_copy(sc_dump[0:1, 0:1], sc["s_x"][0:1, 0:1])
                nc.vector.tensor_copy(sc_dump[0:1, 1:2], sc["ws_k"][0:1, 0:1])
                nc.vector.tensor_copy(sc_dump[0:1, 2:3], sc["sck32"][0:1, 0:1])
                nc.vector.tensor_copy(sc_dump[0:1, 3:4], sc["gmax_x"][0:1, 0:1])
                nc.sync.dma_start(dbg_sc[:], sc_dump[:])
            # mv load rides tile 0: scalar-ring triggers interleave with the
            # exp ops (DMA flows from ~2us into tile 0, keeping phase A's
            # bandwidth clean); drains alternate DVE/ACT, pairs 8..15 first
            mv_parts = []
            for k, mb in enumerate(list(range(8, NM)) + list(range(8))):
                for ch in range(2):
                    def mv_part(k=k, mb=mb, ch=ch):
                        mvt = wp.tile([128, 512], F32, tag="mvt", bufs=2,
                                      name="mvt")
                        nc.scalar.dma_start(
                            mvt[:], mv_in[mb * 128:(mb + 1) * 128,
                                          ch * 512:(ch + 1) * 512])
                        def drain():
                            dst = mv_sb[:, mb, ch * 512:(ch + 1) * 512]
                            if (2 * k + ch) % 2 == 0:
                                nc.vector.tensor_copy(dst, mvt[:])
                            else:
                                nc.scalar.activation(
                                    dst, mvt[:],
                                    mybir.ActivationFunctionType.Copy)
                        return drain
                    mv_parts.append(mv_part)

            for it in range(NT):
                t0 = it * T
                # sims^T -> exp into expT [m_p, mb, t]; denominator partials
                # accumulate on DVE as each exp lands (keeps den off the
                # retr critical path)
                eacc = wp.tile([128, T], F32, tag="eacc", bufs=2, name="eacc")
                qxT_next_buf = qxT_bufs[:, (it + 1) % 2]
                for mb in range(NM):
                    ps = psp.tile([128, T], F32, tag="ps_a", bufs=3,
                                  name="sims_ps")
                    for eb in range(NE):
                        nc.tensor.matmul(
                            ps[:], C_sb[:, eb, mb * 128:(mb + 1) * 128],
                            qxT[:, eb, :], start=(eb == 0), stop=(eb == NE - 1))
                    # exp straight to fp8 from ACT; EXP_SHIFT is chosen so
                    # max exp stays well under 240 (TRN fp8e4 saturates to
                    # Inf above), making the old f16+min(240) hop redundant
                    nc.scalar.activation(
                        expT[:, mb, :], ps[:],
                        mybir.ActivationFunctionType.Exp,
                        bias=rk_sb[:, mb:mb + 1], scale=bc["sck32"][:, 0:1])
                    if mb == 1:
                        nc.vector.tensor_tensor(
                            eacc[:], expT[:, 0, :], expT[:, 1, :],
                            op=mybir.AluOpType.add)
                    elif mb > 1:
                        nc.vector.tensor_tensor(
                            eacc[:], eacc[:], expT[:, mb, :],
                            op=mybir.AluOpType.add)
                    if it == 0:
                        for _ in range(2):
                            mv_parts.pop(0)()()
                    # next tile's qxT quantize interleaves into the back half
                    # of the sims loop so its ACT/DVE ops land before the
                    # epilogue chain floods DVE
                    if mb >= NM - NE and it + 1 < NT:
                        quantize_qxT_eb(qxT_next_buf, it + 1, mb - (NM - NE))

                den_bc = wp.tile([128, T], F32, tag="den_bc", bufs=1, name="den_bc")
                if no_den:
                    nc.vector.tensor_copy(den_bc[:], eacc[:])
                else:
                    nc.gpsimd.partition_all_reduce(den_bc[:], eacc[:], channels=128,
                                                   reduce_op=bass_isa.ReduceOp.add)
                if retr_fp8:
                    # fp8 exp can crush an entire row to 0 -> den=0 -> NaN;
                    # eps keeps inv finite (ret=0 for such rows)
                    nc.vector.tensor_scalar(den_bc[:], den_bc[:], 1e-6, None,
                                            op0=mybir.AluOpType.add)
                inv_bc = wp.tile([128, T], F32, tag="inv_bc", bufs=1, name="inv_bc")
                nc.vector.reciprocal(inv_bc[:], den_bc[:])

                if dbg and it == 0:
                    nc.sync.dma_start(dbg_expT[:],
                                      expT.rearrange("p a b -> p (a b)")[:])
                    nc.sync.dma_start(dbg_inv[:], inv_bc[:])

                # retrieved^T + h^T (in place over xT) + |h| max on Pool
                for eb in range(NE):
                    psr = psp.tile([128, T], F32, tag="ps_r", bufs=3, name="r_ps")
                    if retr_fp8:
                        pairs = [8, 10, 12, 14, 0, 2, 4, 6]
                        for i, mb in enumerate(pairs):
                            nc.tensor.matmul(
                                psr[:],
                                mv_sb[:, mb:mb + 2, eb * 128:(eb + 1) * 128],
                                expT[:, mb:mb + 2, :],
                                start=(i == 0), stop=(i == len(pairs) - 1),
                                perf_mode=mybir.MatmulPerfMode.DoubleRow)
                    else:
                        order = list(range(8, NM)) + list(range(8))
                        for i, mb in enumerate(order):
                            nc.tensor.matmul(
                                psr[:], mv_sb[:, mb, eb * 128:(eb + 1) * 128],
                                expT[:, mb, :],
                                start=(i == 0), stop=(i == NM - 1))
                    # normalize in place in PSUM, then accumulate into hT
                    nc.vector.tensor_tensor(psr[:], psr[:], inv_bc[:],
                                            op=mybir.AluOpType.mult)
                    hsl = hT[:, eb, t0:t0 + T]
                    nc.vector.tensor_tensor(hsl, psr[:], hsl,
                                            op=mybir.AluOpType.add)
                    nc.vector.tensor_reduce(
                        hmax_buf[:, it * NE + eb:it * NE + eb + 1], hsl,
                        axis=mybir.AxisListType.X, op=mybir.AluOpType.max,
                        apply_absolute_value=True)

                # Wo prep slice rides under this tile (idle rings/engines)
                if it in wo_slices:
                    lo = wo_slices.get(it - 1, 0)
                    for w in wo_work[lo:wo_slices[it]]:
                        w()

                if it + 1 < NT:
                    qxT = qxT_next_buf

            if dbg:
                nc.sync.dma_start(dbg_hT[:], hT.rearrange("p a b -> p (a b)")[:])

            wo_transpose()

            # ---- global max|h| -> s_h, output scales ----
            nc.vector.tensor_reduce(hmax_red[:], hmax_buf[:],
                                    axis=mybir.AxisListType.X,
                                    op=mybir.AluOpType.max)
            part_reduce_scalar(hmax_red, sc["red1"], "max")
            allreduce_max(sc["red1"], sc["gmax_h"])
            nc.vector.tensor_scalar(sc["s_h"][0:1, 0:1], sc["gmax_h"][0:1, 0:1],
                                    1.0 / 127.0, None, op0=mybir.AluOpType.mult)
            nc.vector.reciprocal(sc["inv_sh"][0:1, 0:1], sc["s_h"][0:1, 0:1])
            nc.gpsimd.partition_broadcast(bc["inv_sh"][:], sc["inv_sh"][0:1, 0:1])
            nc.vector.tensor_tensor(sc["sco"][0:1, 0:1], sc["ws_o"][0:1, 0:1],
                                    sc["s_h"][0:1, 0:1], op=mybir.AluOpType.mult)
            nc.gpsimd.partition_broadcast(bc["sco"][:], sc["sco"][0:1, 0:1])
            if with_bias:
                nc.vector.reciprocal(sc["inv_sco"][0:1, 0:1], sc["sco"][0:1, 0:1])
                nc.vector.tensor_scalar(bo_sc[0:1, :], bo_sb[0:1, :],
                                        sc["inv_sco"][0:1, 0:1], None,
                                        op0=mybir.AluOpType.mult)

            # ======================= PHASE C =======================
            def quantize_h(it):
                # in-place rne(hT/s_h); magic-round alternates ACT/DVE to
                # halve the serial latency at the B->C transition
                for eb in range(NE):
                    hsl = hT[:, eb, it * T:(it + 1) * T]
                    qt = wp.tile([128, T], F32, tag="inv_bc", bufs=1, name="qh")
                    if eb % 2 == 0:
                        nc.scalar.activation(
                            qt[:], hsl, mybir.ActivationFunctionType.Identity,
                            bias=magic_bc[:, 0:1], scale=bc["inv_sh"][:, 0:1])
                    else:
                        nc.vector.tensor_scalar(
                            qt[:], hsl, bc["inv_sh"][:, 0:1], magic_bc[:, 0:1],
                            op0=mybir.AluOpType.mult, op1=mybir.AluOpType.add)
                    nc.vector.tensor_scalar(hsl, qt[:], MAGIC, None,
                                            op0=mybir.AluOpType.subtract)

            quantize_h(0)
            for it in range(NT):
                t0 = it * T
                if it + 1 < NT:
                    quantize_h(it + 1)
                for tsub in range(TS):
                    tc0 = t0 + tsub * 128
                    # both output halves accumulate together so consecutive
                    # matmuls share the same lhsT (Ldweights dedup)
                    opss = [psp.tile([128, 512], F32, tag="ps_a", bufs=3,
                                     name="o_ps") for _ in range(NOH)]
                    for eb in range(NE):
                        for oh in range(NOH):
                            of = slice(oh * 512, (oh + 1) * 512)
                            nc.tensor.matmul(
                                opss[oh][:], hT[:, eb, tc0:tc0 + 128],
                                qWo[:, eb, of],
                                start=(eb == 0),
                                stop=(not with_bias and eb == NE - 1))
                    for oh in range(NOH):
                        of = slice(oh * 512, (oh + 1) * 512)
                        if with_bias:
                            nc.tensor.matmul(opss[oh][:], ones_row[0:1, :],
                                             bo_sc[0:1, of],
                                             start=False, stop=True)
                        osb = wp.tile([128, 512], F32, tag="eacc", bufs=2,
                                      name="osb")
                        nc.scalar.activation(
                            osb[:], opss[oh][:],
                            mybir.ActivationFunctionType.Copy,
                            bias=0.0, scale=bc["sco"][:, 0:1])
                        nc.scalar.dma_start(out_ext[tc0:tc0 + 128, of], osb[:])

    nc.compile()
    if dedup_ldw:
        _dedup_ldweights(nc)
    return nc


def _dedup_ldweights(nc):
    """Remove an InstLdweights identical to the previous one when only
    matmuls/event-semaphores sit between (weights already resident).
    Its waits move onto the following matmul."""
    removed = 0
    for blk in nc.m.functions[0].blocks:
        insts = list(blk.instructions)
        last_key = None
        drop = []          # indices to delete
        carry = None       # waits carried from a dropped LDW
        for i, inst in enumerate(insts):
            tn = type(inst).__name__
            if tn == "InstLdweights":
                pap = inst.ins[0]
                key = (getattr(pap, "memref", None), getattr(pap, "offset", None),
                       str(getattr(pap, "ap", None)), str(getattr(pap, "dtype", None)))
                si = inst.sync_info
                ups = list(si.on_update) if si else []
                if key == last_key and not ups:
                    drop.append(i)
                    if si and len(list(si.on_wait)):
                        carry = (list(si.on_wait), i)
                else:
                    last_key = key
            elif tn == "InstMatmult":
                if carry is not None:
                    w, _ = carry
                    si = inst.sync_info
                    if si is None:
                        inst.sync_info = mybir.SyncInfo(on_wait=w, on_update=[])
                    else:
                        si.on_wait = list(si.on_wait) + w
                    carry = None
            elif tn == "InstEventSemaphore":
                pass
            else:
                last_key = None
                if carry is not None:   # shouldn't happen; keep the LDW then
                    drop.remove(carry[1])
                    carry = None
        for i in reversed(drop):
            del blk.instructions[i]
        removed += len(drop)
    return removed


# ----------------------------------------------------------------------------
_CACHE = {}


def _get_nc(key="full", **kw):
    if key not in _CACHE:
        _CACHE[key] = build_nc(**kw)
    return _CACHE[key]


def _make_in_maps(x, memory_keys, memory_values, Wk, bk, Wo, bo):
    x = np.ascontiguousarray(x, dtype=np.float32)
    shared = {
        "memory_keys": np.ascontiguousarray(memory_keys, dtype=np.float32),
        "memory_values": np.ascontiguousarray(memory_values, dtype=np.float32),
        "Wk": np.ascontiguousarray(Wk, dtype=np.float32),
        "bk": np.ascontiguousarray(bk, dtype=np.float32),
        "Wo": np.ascontiguousarray(Wo, dtype=np.float32),
        "bo": np.ascontiguousarray(bo, dtype=np.float32),
    }
    return [dict(shared, x=x[i]) for i in range(x.shape[0])]


def kernel(x, memory_keys, memory_values, Wk, bk, Wv=None, bv=None, Wo=None, bo=None):
    wb = bool(np.any(np.asarray(bo)))
    wk_b = bool(np.any(np.asarray(bk)))
    nc = _get_nc(("full", wb, wk_b), with_bias=wb, with_bk=wk_b)
    in_maps = _make_in_maps(x, memory_keys, memory_values, Wk, bk, Wo, bo)
    res = run_bass_kernel_spmd(nc, in_maps, core_ids=list(range(N_CORES)))
    out = np.stack([res.results[i]["out"] for i in range(N_CORES)], axis=0)
    return out.astype(np.float32)


# ------------------------- benchmarking helper ------------------------------
def bench(inputs, iters=5, nc=None):
    """Time on-device execution with device-resident inputs."""
    import jax
    from jax.sharding import Mesh, PartitionSpec, NamedSharding
    from jax.experimental.shard_map import shard_map
    from concourse import bass2jax as b2j

    if nc is None:
        wb = bool(np.any(np.asarray(inputs["bo"])))
        wk_b = bool(np.any(np.asarray(inputs["bk"])))
        nc = _get_nc(("full", wb, wk_b), with_bias=wb, with_bk=wk_b)
    in_maps = _make_in_maps(inputs["x"], inputs["memory_keys"],
                            inputs["memory_values"], inputs["Wk"], inputs["bk"],
                            inputs["Wo"], inputs["bo"])
    b2j.install_neuronx_cc_hook()

    partition_name = nc.partition_id_tensor.name if nc.partition_id_tensor else None
    in_names, out_names, out_avals, zero_outs = [], [], [], []
    for alloc in nc.m.functions[0].allocations:
        if not isinstance(alloc, mybir.MemoryLocationSet):
            continue
        name = alloc.memorylocations[0].name
        if alloc.kind == "ExternalInput":
            if name != partition_name:
                in_names.append(name)
        elif alloc.kind == "ExternalOutput":
            out_names.append(name)
            shape = tuple(alloc.tensor_shape)
            dtype = mybir.dt.np(alloc.dtype)
            out_avals.append(jax.core.ShapedArray(shape, dtype))
            zero_outs.append(np.zeros(shape, dtype))
    n_params = len(in_names)
    n_outs = len(out_avals)
    in_names = in_names + out_names
    if partition_name is not None:
        in_names.append(partition_name)

    def _body(*args):
        operands = list(args)
        if partition_name is not None:
            operands.append(b2j.partition_id_tensor())
        outs = b2j._bass_exec_p.bind(
            *operands, out_avals=tuple(out_avals), in_names=tuple(in_names),
            out_names=tuple(out_names), lowering_input_output_aliases=(),
            sim_require_finite=True, sim_require_nnan=True, nc=nc)
        return tuple(outs)

    n_cores = len(in_maps)
    devices = jax.devices()[:n_cores]
    mesh = Mesh(np.asarray(devices), ("core",))
    in_specs = (PartitionSpec("core"),) * (n_params + n_outs)
    out_specs = (PartitionSpec("core"),) * len(out_names)
    donate = tuple(range(n_params, n_params + n_outs))
    sharded = jax.jit(
        shard_map(_body, mesh=mesh, in_specs=in_specs, out_specs=out_specs,
                  check_rep=False),
        donate_argnums=donate, keep_unused=True)

    per_core = [[np.asarray(m[nm]) for nm in in_names[:n_params]] for m in in_maps]
    concat_in = [np.concatenate([per_core[c][i] for c in range(n_cores)], axis=0)
                 for i in range(n_params)]
    sh = NamedSharding(mesh, PartitionSpec("core"))
    dev_in = [jax.device_put(a, sh) for a in concat_in]
    for a in dev_in:
        a.block_until_ready()

    times = []
    out_arrs = None
    for i in range(iters + 1):
        dev_zeros = [jax.device_put(
            np.zeros((n_cores * z.shape[0], *z.shape[1:]), z.dtype), sh)
            for z in zero_outs]
        for a in dev_zeros:
            a.block_until_ready()
        t0 = time.perf_counter()
        out_arrs = sharded(*dev_in, *dev_zeros)
        for o in out_arrs:
            o.block_until_ready()
        t1 = time.perf_counter()
        if i > 0:
            times.append(t1 - t0)
    oi = out_names.index("out")
    oshape = out_avals[oi].shape
    out = np.asarray(out_arrs[oi]).reshape(n_cores, *oshape)
    return times, out



# revision 54
# speedup vs baseline: 1.3272x; 1.3272x over previous
"""Trainium2 Bass kernel for nn_EpisodicMemory (BitNet projections + memory cross-attention).

kernel(**inputs) takes FULL unsharded numpy inputs, returns FULL output
[8, 4096, 1024] f32. Batch-parallel across 8 NeuronCores; two scalar
AllReduce(max) collectives provide the global BitNet activation scales.

Design ("folded + transposed dataflow"):
  - Fold the key projection into the memory bank:
        C = qWk^T @ mk^T   [E, M]     (computed in phase A, PE otherwise idle)
        sims = s_ck * (qx @ C) + bk@mk^T
    eliminating the qk matmul from the per-tile critical path.
  - Keep x TRANSPOSED on-chip: x is cast to fp16 and PE-transposed into a
    resident SBUF buffer during the phase-A streaming pass (abs-max on the
    way), so qxT = rne(xT/s_x) is elementwise, and
        simsT[m,t]: lhsT=C[e,mcols], rhs=qxT[e,t]
        retT[e,t]:  lhsT=mv[m,ecols] (natural!), rhs=expT[m,t]
        hT = xT + retT/den  (in place, same SBUF buffer)
        out[t,e]:   lhsT=qhT[e,tcols], rhs=qWoT[e,eout]
    so NO h spill, NO qx bounce, NO DRAM transposes at all.
  - Denominator via DVE adds + gpsimd partition_all_reduce (no PE cost).
  - Retrieval matmul runs in fp8e4 DoubleRow (2 k-subtiles per matmul):
    exp values are clipped at 240 (TRN fp8e4 saturates to Inf above) and
    the denominator gets +1e-6 so fully-underflowed rows yield ret=0, not
    NaN. sims/out matmuls stay fp16 (fp8 there exceeds the error budget).
  - Sims/out matmuls fp16 with fp32 PSUM accumulation (BitNet ones exact).
"""

import math
import time

import numpy as np

import concourse.bass as bass
import concourse.tile as tile
from concourse import bacc, bass_isa, mybir
from concourse.bass_utils import run_bass_kernel_spmd

F32 = mybir.dt.float32
F16 = mybir.dt.float16
F8 = mybir.dt.float8e4

N_CORES = 8
MAGIC = 1.5 * (2.0 ** 23)   # fp32 RNE rounding trick
EXP_SHIFT = 8.3             # logit shift: max exp(logit-shift) < 240 (fp8e4 Inf bound)

B, S_FULL, E_DIM, DM_DIM, M_DIM = 8, 4096, 1024, 1024, 2048


def build_nc(S=S_FULL, E=E_DIM, DM=DM_DIM, M=M_DIM, T=512, repeat=1,
             use_collectives=True, with_bias=True, with_bk=False, dbg=False,
             retr_fp8=True, no_den=False, dedup_ldw=False):
    assert S % T == 0 and T % 128 == 0 and E == 1024 and DM == 1024 and M == 2048
    exp_shift = EXP_SHIFT
    NT = S // T          # 8 row tiles
    TS = T // 128        # 4
    NE = E // 128        # 8
    ND = DM // 128       # 8
    NM = M // 128        # 16
    NOH = E // 512       # 2
    NXC = (S // 128) * 2  # 64 x half-chunks

    nc = bacc.Bacc("TRN2", target_bir_lowering=False, debug=False,
                   num_devices=N_CORES)

    x_in = nc.dram_tensor("x", [S, E], F32, kind="ExternalInput").ap()
    mk_in = nc.dram_tensor("memory_keys", [M, DM], F32, kind="ExternalInput").ap()
    mv_in = nc.dram_tensor("memory_values", [M, E], F32, kind="ExternalInput").ap()
    wk_in = nc.dram_tensor("Wk", [DM, E], F32, kind="ExternalInput").ap()
    bk_in = nc.dram_tensor("bk", [DM], F32, kind="ExternalInput").ap()
    wo_in = nc.dram_tensor("Wo", [E, E], F32, kind="ExternalInput").ap()
    bo_in = nc.dram_tensor("bo", [E], F32, kind="ExternalInput").ap()
    out_ext = nc.dram_tensor("out", [S, E], F32, kind="ExternalOutput").ap()
    if dbg:
        dbg_C = nc.dram_tensor("dbg_C", [128, 8 * M], F16, kind="ExternalOutput").ap()
        dbg_qxT = nc.dram_tensor("dbg_qxT", [128, 8 * T], F16, kind="ExternalOutput").ap()
        dbg_expT = nc.dram_tensor("dbg_expT", [128, 16 * T], F8 if retr_fp8 else F16, kind="ExternalOutput").ap()
        dbg_inv = nc.dram_tensor("dbg_inv", [128, T], F32, kind="ExternalOutput").ap()
        dbg_hT = nc.dram_tensor("dbg_hT", [128, 8 * S], F16, kind="ExternalOutput").ap()
        dbg_sc = nc.dram_tensor("dbg_sc", [1, 8], F32, kind="ExternalOutput").ap()

    import contextlib
    with tile.TileContext(nc) as tc:
        # pools OUTSIDE the repeat loop: no pool-drain barrier per iteration,
        # so consecutive iterations pipeline (next input stream under current
        # output matmuls); loop-carried WAR deps come from Tile semaphores
        with (
            tc.tile_pool(name="pp", bufs=1) as pp,
            tc.tile_pool(name="wp", bufs=2) as wp,
            tc.tile_pool(name="psp", bufs=2, space="PSUM") as psp,
            tc.tile_pool(name="dp", bufs=1, space="DRAM") as dp,
        ):
          loop_cm = tc.For_i(0, repeat, 1) if repeat > 1 else contextlib.nullcontext()
          with loop_cm:
            # ---------------- persistent SBUF ----------------
            EV = F8 if retr_fp8 else F16
            hT = pp.tile([128, NE, S], F16, tag="hT")        # xT then hT, 64KB/p
            C_sb = pp.tile([128, NE, M], F16, tag="C_sb")    # 32KB/p
            mv_sb = pp.tile([128, NM, E], EV, tag="mv_sb")   # 16/32KB/p
            qWk = pp.tile([128, ND, E], F16, tag="qWk")      # 16KB/p (phase A)
            qWo = pp.tile([128, NE, E], F16, tag="qWo")      # 16KB/p
            big2 = pp.tile([128, ND, T], F16, tag="big2")    # mkT chunk, 8KB/p
            if retr_fp8:
                expT = pp.tile([128, NM, T], F8, tag="expT")  # 8KB/p
            else:
                # fp16 fallback (debug only; qWk now hosts the qxT buffers)
                expT = pp.tile([128, NM, T], F16, tag="expT16")

            rk_sb = pp.tile([128, NM], F32, tag="rk_sb")     # rk/32 - 8 per mb
            bk_sb = pp.tile([128, ND], F32, tag="bk_sb")
            bk16 = pp.tile([128, ND], F16, tag="bk16")
            bo_sb = pp.tile([1, E], F32, tag="bo_sb")
            bo_sc = pp.tile([1, E], F16, tag="bo_sc")
            ones_row = pp.tile([1, 128], F16, tag="ones_row")
            ident16 = pp.tile([128, 128], F16, tag="ident16")
            xmax_buf = pp.tile([128, NXC], F32, tag="xmax_buf")
            hmax_buf = pp.tile([128, NT * NE], F32, tag="hmax_buf")
            xmax_red = pp.tile([128, 1], F32, tag="xmax_red")
            hmax_red = pp.tile([128, 1], F32, tag="hmax_red")
            pr_max = pp.tile([128, 1], F32, tag="pr_max")
            pr_add = pp.tile([128, 1], F32, tag="pr_add")

            sc = {}
            for nm in ("gmax_x", "s_x", "inv_sx", "ws_k", "thr_k", "nthr_k",
                       "ws_o", "thr_o", "nthr_o", "sck32", "gmax_h", "s_h",
                       "inv_sh", "sco", "inv_sco", "red1", "red2", "inv_wso",
                       "inv_wsk"):
                sc[nm] = pp.tile([1, 8], F32, name=f"sc_{nm}", tag=f"sc_{nm}")
            bc = {}
            for nm in ("inv_sx", "thr_k", "nthr_k", "thr_o", "nthr_o", "sck32",
                       "inv_sh", "sco", "inv_wso", "inv_wsk"):
                bc[nm] = pp.tile([128, 1], F32, name=f"bc_{nm}", tag=f"bc_{nm}")

            magic_bc = pp.tile([128, 1], F32, tag="magic_bc")
            nc.vector.memset(magic_bc[:], MAGIC)
            nc.vector.memset(ones_row[:], 1.0)
            from concourse.masks import make_identity
            make_identity(nc, ident16[:])

            rg = [list(range(N_CORES))]

            def allreduce_max(src_scalar, dst_scalar):
                if not use_collectives:
                    nc.vector.tensor_copy(dst_scalar[0:1, 0:1], src_scalar[0:1, 0:1])
                    return
                ccin = dp.tile([1, 8], F32, tag="cc_in", bufs=2, name="ccin")
                ccout = dp.tile([1, 8], F32, addr_space="Shared", tag="cc_out",
                                bufs=2, name="ccout")
                nc.sync.dma_start(ccin[:], src_scalar[:])
                nc.gpsimd.collective_compute(
                    "AllReduce", mybir.AluOpType.max, replica_groups=rg,
                    ins=[ccin[:]], outs=[ccout[:]])
                nc.sync.dma_start(dst_scalar[:], ccout[:])

            def part_reduce_scalar(vec128, out_scalar, op):
                red = bass_isa.ReduceOp.max if op == "max" else bass_isa.ReduceOp.add
                dst = pr_max if op == "max" else pr_add
                nc.gpsimd.partition_all_reduce(dst[:], vec128[:], channels=128,
                                               reduce_op=red)
                nc.vector.tensor_copy(out_scalar[0:1, 0:1], dst[0:1, 0:1])

            # ============ PHASE A ============
            # gpsimd ring: bk, Wk (2-pass), mk chunks, [mv after C], Wo in B
            # sync ring: x chunk loads + xT transposes; scalar ring: xf16 stores
            nc.gpsimd.dma_start(bk_sb[:], bk_in.rearrange("(b p) -> p b", p=128))
            nc.vector.tensor_copy(bk16[:], bk_sb[:])
            nc.gpsimd.dma_start(bo_sb[0:1, :],
                                bo_in.rearrange("(a e) -> a e", a=1))

            # ---- Wk pass 1: mean|Wk| (scalar ring; DVE reduces) ----
            wk_acc = wp.tile([128, 16], F32, tag="wk_acc", bufs=1, name="wk_acc")
            for i in range(16):
                wt = wp.tile([128, 512], F32, tag="wt", bufs=2, name="wt")
                nc.scalar.dma_start(
                    wt[:], wk_in[(i // 2) * 128:(i // 2 + 1) * 128,
                                 (i % 2) * 512:(i % 2) * 512 + 512])
                nc.vector.tensor_reduce(wk_acc[:, i:i + 1], wt[:],
                                        axis=mybir.AxisListType.X,
                                        op=mybir.AluOpType.add,
                                        apply_absolute_value=True)
            wk_accr = wp.tile([128, 1], F32, tag="wk_accr", bufs=1, name="wk_accr")
            nc.vector.tensor_reduce(wk_accr[:], wk_acc[:], axis=mybir.AxisListType.X,
                                    op=mybir.AluOpType.add)
            part_reduce_scalar(wk_accr, sc["red2"], "add")
            nc.vector.tensor_scalar(sc["ws_k"][0:1, 0:1], sc["red2"][0:1, 0:1],
                                    1.0 / (DM * E), None, op0=mybir.AluOpType.mult)
            nc.vector.reciprocal(sc["inv_wsk"][0:1, 0:1], sc["ws_k"][0:1, 0:1])
            nc.gpsimd.partition_broadcast(bc["inv_wsk"][:],
                                          sc["inv_wsk"][0:1, 0:1])

            # ---- helpers for the interleaved phase A main loop ----
            def x_chunk(cx):
                # abs-max + cast fp16 + PE-transpose into hT (grouped copies)
                xc16 = wp.tile([128, E], F16, tag="c16", bufs=2, name="xc16")
                for hf in range(2):
                    cf = slice(hf * 512, (hf + 1) * 512)
                    xc32 = wp.tile([128, 512], F32, tag="xc32", bufs=3, name="xc32")
                    nc.sync.dma_start(xc32[:], x_in[cx * 128:(cx + 1) * 128, cf])
                    nc.vector.tensor_reduce(xmax_buf[:, cx * 2 + hf:cx * 2 + hf + 1],
                                            xc32[:],
                                            axis=mybir.AxisListType.X,
                                            op=mybir.AluOpType.max,
                                            apply_absolute_value=True)
                    if hf == 0:
                        nc.scalar.activation(xc16[:, cf], xc32[:],
                                             mybir.ActivationFunctionType.Copy)
                    else:
                        nc.vector.tensor_copy(xc16[:, cf], xc32[:])
                for g in range(2):
                    tpg = psp.tile([128, 512], F16, tag="ps_x", bufs=2, name="x_ps")
                    for j in range(4):
                        eb = g * 4 + j
                        nc.tensor.transpose(tpg[:, j * 128:(j + 1) * 128],
                                            xc16[:, eb * 128:(eb + 1) * 128],
                                            ident16[:])
                    dst = hT[:, g * 4:(g + 1) * 4, cx * 128:(cx + 1) * 128]
                    if g == 0:
                        nc.scalar.activation(dst, tpg[:],
                                             mybir.ActivationFunctionType.Copy)
                    else:
                        nc.vector.tensor_copy(dst, tpg[:])

            def wk_pass2_chunk(i):
                # ternarize = clip(rne(w/ws_k), -1, 1): ACT magic-round +
                # DVE un-bias/clip (no compare ops)
                db, ch = i // 2, i % 2
                wt = wp.tile([128, 512], F32, tag="wt", bufs=2, name="wt2")
                nc.scalar.dma_start(
                    wt[:], wk_in[db * 128:(db + 1) * 128,
                                 ch * 512:(ch + 1) * 512])
                t = wp.tile([128, 512], F32, tag="f32s", bufs=2, name="wk_t")
                nc.scalar.activation(
                    t[:], wt[:], mybir.ActivationFunctionType.Identity,
                    bias=magic_bc[:, 0:1], scale=bc["inv_wsk"][:, 0:1])
                q = wp.tile([128, 512], F16, tag="wk_q", bufs=1, name="wk_q")
                nc.vector.tensor_scalar(q[:], t[:], MAGIC, None,
                                        op0=mybir.AluOpType.subtract)
                nc.vector.tensor_scalar(qWk[:, db, ch * 512:(ch + 1) * 512],
                                        q[:], 1.0, -1.0,
                                        op0=mybir.AluOpType.min,
                                        op1=mybir.AluOpType.max)

            def mk_block(blk):
                mrow = blk * 128
                mbb = blk % 4
                mk16 = wp.tile([128, E], F16, tag="mk16", bufs=2, name="mk16")
                for ch in range(2):
                    mkt = wp.tile([128, 512], F32, tag="mvt", bufs=3, name="mkt")
                    nc.scalar.dma_start(
                        mkt[:], mk_in[mrow:mrow + 128, ch * 512:(ch + 1) * 512])
                    if ch == 0:
                        nc.scalar.activation(mk16[:, ch * 512:(ch + 1) * 512],
                                             mkt[:],
                                             mybir.ActivationFunctionType.Copy)
                    else:
                        nc.vector.tensor_copy(mk16[:, ch * 512:(ch + 1) * 512],
                                              mkt[:])
                for g in range(2):
                    tpg = psp.tile([128, 512], F16, tag="ps_x", bufs=2, name="mk_ps")
                    for j in range(4):
                        db = g * 4 + j
                        nc.tensor.transpose(tpg[:, j * 128:(j + 1) * 128],
                                            mk16[:, db * 128:(db + 1) * 128],
                                            ident16[:])
                    dst = big2[:, g * 4:(g + 1) * 4, mbb * 128:(mbb + 1) * 128]
                    if g == 0:
                        nc.scalar.activation(dst, tpg[:],
                                             mybir.ActivationFunctionType.Copy)
                    else:
                        nc.vector.tensor_copy(dst, tpg[:])

            def c_chunk(mch):
                for eb in range(NE):
                    psc = psp.tile([128, 512], F32, tag="ps_r", bufs=3,
                                   name="c_ps")
                    for db in range(ND):
                        nc.tensor.matmul(
                            psc[:], qWk[:, db, eb * 128:(eb + 1) * 128],
                            big2[:, db, :],
                            start=(db == 0), stop=(db == ND - 1))
                    if eb % 2 == 0:
                        nc.scalar.activation(
                            C_sb[:, eb, mch * 512:(mch + 1) * 512], psc[:],
                            mybir.ActivationFunctionType.Copy)
                    else:
                        nc.vector.tensor_copy(
                            C_sb[:, eb, mch * 512:(mch + 1) * 512], psc[:])
                if with_bk:
                    for mbb in range(4):
                        mb = mch * 4 + mbb
                        psk = psp.tile([128, 8], F32, tag="ps_k", name="rk_ps")
                        for db in range(ND):
                            nc.tensor.matmul(
                                psk[:, 0:1],
                                big2[:, db, mbb * 128:(mbb + 1) * 128],
                                bk16[:, db:db + 1],
                                start=(db == 0), stop=(db == ND - 1))
                        nc.scalar.activation(
                            rk_sb[:, mb:mb + 1], psk[:, 0:1],
                            mybir.ActivationFunctionType.Copy,
                            bias=-exp_shift, scale=1.0 / math.sqrt(DM))

            # ---- interleaved phase A main loop ----
            if not with_bk:
                nc.vector.memset(rk_sb[:], -exp_shift)
            for i in range(S // 128):
                if 1 <= i < 9:
                    wk_pass2_chunk(2 * (i - 1))
                    wk_pass2_chunk(2 * (i - 1) + 1)
                if i in (10, 14, 18, 22):
                    c_chunk((i - 10) // 4)  # before mk_block overwrites big2
                if 6 <= i < 22:
                    mk_block(i - 6)
                x_chunk(i)

            nc.vector.tensor_reduce(xmax_red[:], xmax_buf[:],
                                    axis=mybir.AxisListType.X,
                                    op=mybir.AluOpType.max)
            part_reduce_scalar(xmax_red, sc["red1"], "max")
            allreduce_max(sc["red1"], sc["gmax_x"])
            nc.vector.tensor_scalar(sc["s_x"][0:1, 0:1], sc["gmax_x"][0:1, 0:1],
                                    1.0 / 127.0, None, op0=mybir.AluOpType.mult)
            nc.vector.reciprocal(sc["inv_sx"][0:1, 0:1], sc["s_x"][0:1, 0:1])
            nc.gpsimd.partition_broadcast(bc["inv_sx"][:], sc["inv_sx"][0:1, 0:1])
            # sck32 = ws_k * s_x / sqrt(DM)
            nc.vector.tensor_tensor(sc["sck32"][0:1, 0:1], sc["ws_k"][0:1, 0:1],
                                    sc["s_x"][0:1, 0:1], op=mybir.AluOpType.mult)
            nc.vector.tensor_scalar(sc["sck32"][0:1, 0:1], sc["sck32"][0:1, 0:1],
                                    1.0 / math.sqrt(DM), None,
                                    op0=mybir.AluOpType.mult)
            nc.gpsimd.partition_broadcast(bc["sck32"][:], sc["sck32"][0:1, 0:1])

            # ======================= PHASE B =======================
            # qxT double-buffer lives in qWk's SBUF (dead after C is built):
            # [128, ND, E] f16 -> [128, 2, NE, T] f16
            qxT_bufs = qWk.rearrange("p a (b c) -> p (a b) c", b=2, c=T) \
                          .rearrange("p (z e) c -> p z e c", z=2, e=NE)

            def quantize_qxT_eb(qxT, it, eb):
                # magic-round split across ACT (mul+add) and DVE (sub)
                qt = wp.tile([128, T], F32, tag="wt", bufs=2, name="qt")
                nc.scalar.activation(
                    qt[:], hT[:, eb, it * T:(it + 1) * T],
                    mybir.ActivationFunctionType.Identity,
                    bias=magic_bc[:, 0:1], scale=bc["inv_sx"][:, 0:1])
                nc.vector.tensor_scalar(
                    qxT[:, eb, :], qt[:], MAGIC, None,
                    op0=mybir.AluOpType.subtract)

            def quantize_qxT(it):
                qxT = qxT_bufs[:, it % 2]
                for eb in range(NE):
                    quantize_qxT_eb(qxT, it, eb)
                return qxT

            # Wo 2-pass ternarize into qWo, sliced into closures so the work
            # spreads across phase-B tiles 1-5 (DMA on the otherwise-idle
            # gpsimd/scalar rings, DVE/PE crumbs under each tile)
            wo_acc = wp.tile([128, 16], F32, tag="wk_acc", bufs=1, name="wo_acc")

            def wo_p1(i):
                wt = wp.tile([128, 512], F32, tag="mvt", bufs=3, name="wot")
                nc.gpsimd.dma_start(
                    wt[:], wo_in[(i // 2) * 128:(i // 2 + 1) * 128,
                                 (i % 2) * 512:(i % 2) * 512 + 512])
                nc.vector.tensor_reduce(wo_acc[:, i:i + 1], wt[:],
                                        axis=mybir.AxisListType.X,
                                        op=mybir.AluOpType.add,
                                        apply_absolute_value=True)

            def wo_thr():
                wo_accr = wp.tile([128, 1], F32, tag="wk_accr", bufs=1,
                                  name="wo_accr")
                nc.vector.tensor_reduce(wo_accr[:], wo_acc[:],
                                        axis=mybir.AxisListType.X,
                                        op=mybir.AluOpType.add)
                part_reduce_scalar(wo_accr, sc["red2"], "add")
                nc.vector.tensor_scalar(sc["ws_o"][0:1, 0:1], sc["red2"][0:1, 0:1],
                                        1.0 / (E * E), None,
                                        op0=mybir.AluOpType.mult)
                nc.vector.reciprocal(sc["inv_wso"][0:1, 0:1], sc["ws_o"][0:1, 0:1])
                nc.gpsimd.partition_broadcast(bc["inv_wso"][:],
                                              sc["inv_wso"][0:1, 0:1])

            def wo_p2(ob, ch):
                # ternarize = clip(rne(w/ws_o), -1, 1): ACT magic-round,
                # Pool un-bias + clip -- zero DVE/PE cost under phase-B
                # tiles. Raw (untransposed) block parks in its own qWo
                # region; wo_transpose fixes the layout at the B->C gap.
                wt = wp.tile([128, 512], F32, tag="mvt", bufs=3, name="wot2")
                nc.scalar.dma_start(
                    wt[:], wo_in[ob * 128:(ob + 1) * 128,
                                 ch * 512:(ch + 1) * 512])
                t = wp.tile([128, 512], F32, tag="f32s", bufs=2, name="wo_t")
                nc.scalar.activation(
                    t[:], wt[:], mybir.ActivationFunctionType.Identity,
                    bias=magic_bc[:, 0:1], scale=bc["inv_wso"][:, 0:1])
                q = wp.tile([128, 512], F32, tag="f32s", bufs=2, name="wo_q")
                nc.gpsimd.tensor_scalar(q[:], t[:], MAGIC, None,
                                        op0=mybir.AluOpType.subtract)
                nc.gpsimd.tensor_scalar(
                    qWo[:, ch * 4:(ch + 1) * 4, ob * 128:(ob + 1) * 128],
                    q[:], 1.0, -1.0,
                    op0=mybir.AluOpType.min, op1=mybir.AluOpType.max)

            def wo_transpose():
                # in-place per-region transpose into phase C rhs layout
                # [i_p, ib, o]; runs in the B->C hmax/allreduce shadow
                for ob in range(NE):
                    for ch in range(2):
                        reg = qWo[:, ch * 4:(ch + 1) * 4,
                                  ob * 128:(ob + 1) * 128]
                        tpg = psp.tile([128, 512], F16, tag="ps_x", bufs=2,
                                       name="wo_ps")
                        for j in range(4):
                            nc.tensor.transpose(
                                tpg[:, j * 128:(j + 1) * 128],
                                qWo[:, ch * 4 + j, ob * 128:(ob + 1) * 128],
                                ident16[:])
                        if (ob + ch) % 2 == 0:
                            nc.scalar.activation(
                                reg, tpg[:],
                                mybir.ActivationFunctionType.Copy)
                        else:
                            nc.vector.tensor_copy(reg, tpg[:])

            wo_work = (
                [(lambda i=i: wo_p1(i)) for i in range(16)]
                + [wo_thr]
                + [(lambda ob=ob, ch=ch: wo_p2(ob, ch))
                   for ob in range(NE) for ch in range(2)]
            )
            # cumulative slice boundaries per phase-B tile index (tiles 1..6)
            wo_slices = {1: 8, 2: 17, 3: 21, 4: 25, 5: 29, 6: 33}
            assert wo_slices[6] == len(wo_work)

            qxT = quantize_qxT(0)
            if dbg:
                nc.sync.dma_start(dbg_C[:], C_sb.rearrange("p a b -> p (a b)")[:])
                nc.sync.dma_start(dbg_qxT[:], qxT.rearrange("p a b -> p (a b)")[:])
                sc_dump = pp.tile([1, 8], F32, tag="sc_dump")
                nc.vector.memset(sc_dump[:], 0.0)
                nc.vector.tensor_copy(sc_dump[0:1, 0:1], sc["s_x"][0:1, 0:1])
                nc.vector.tensor_copy(sc_dump[0:1, 1:2], sc["ws_k"][0:1, 0:1])
                nc.vector.tensor_copy(sc_dump[0:1, 2:3], sc["sck32"][0:1, 0:1])
                nc.vector.tensor_copy(sc_dump[0:1, 3:4], sc["gmax_x"][0:1, 0:1])
                nc.sync.dma_start(dbg_sc[:], sc_dump[:])
            # mv load rides tile 0: scalar-ring triggers interleave with the
            # exp ops (DMA flows from ~2us into tile 0, keeping phase A's
            # bandwidth clean); drains alternate DVE/ACT, pairs 8..15 first
            mv_parts = []
            for k, mb in enumerate(list(range(8, NM)) + list(range(8))):
                for ch in range(2):
                    def mv_part(k=k, mb=mb, ch=ch):
                        mvt = wp.tile([128, 512], F32, tag="mvt", bufs=3,
                                      name="mvt")
                        nc.scalar.dma_start(
                            mvt[:], mv_in[mb * 128:(mb + 1) * 128,
                                          ch * 512:(ch + 1) * 512])
                        def drain():
                            dst = mv_sb[:, mb, ch * 512:(ch + 1) * 512]
                            if (2 * k + ch) % 2 == 0:
                                nc.vector.tensor_copy(dst, mvt[:])
                            else:
                                nc.scalar.activation(
                                    dst, mvt[:],
                                    mybir.ActivationFunctionType.Copy)
                        return drain
                    mv_parts.append(mv_part)

            for it in range(NT):
                t0 = it * T
                # sims^T -> exp into expT [m_p, mb, t]; denominator partials
                # accumulate on DVE as each exp lands (keeps den off the
                # retr critical path)
                eacc = wp.tile([128, T], F32, tag="eacc", bufs=2, name="eacc")
                qxT_next_buf = qxT_bufs[:, (it + 1) % 2]
                for mb in range(NM):
                    ps = psp.tile([128, T], F32, tag="ps_a", bufs=3,
                                  name="sims_ps")
                    for eb in range(NE):
                        nc.tensor.matmul(
                            ps[:], C_sb[:, eb, mb * 128:(mb + 1) * 128],
                            qxT[:, eb, :], start=(eb == 0), stop=(eb == NE - 1))
                    # exp straight to fp8 from ACT; EXP_SHIFT is chosen so
                    # max exp stays well under 240 (TRN fp8e4 saturates to
                    # Inf above), making the old f16+min(240) hop redundant
                    nc.scalar.activation(
                        expT[:, mb, :], ps[:],
                        mybir.ActivationFunctionType.Exp,
                        bias=rk_sb[:, mb:mb + 1], scale=bc["sck32"][:, 0:1])
                    if mb == 1:
                        nc.vector.tensor_tensor(
                            eacc[:], expT[:, 0, :], expT[:, 1, :],
                            op=mybir.AluOpType.add)
                    elif mb > 1:
                        nc.vector.tensor_tensor(
                            eacc[:], eacc[:], expT[:, mb, :],
                            op=mybir.AluOpType.add)
                    if it == 0:
                        for _ in range(2):
                            mv_parts.pop(0)()()
                    # next tile's qxT quantize interleaves into the back half
                    # of the sims loop so its ACT/DVE ops land before the
                    # epilogue chain floods DVE
                    if mb >= NM - NE and it + 1 < NT:
                        quantize_qxT_eb(qxT_next_buf, it + 1, mb - (NM - NE))

                den_bc = wp.tile([128, T], F32, tag="den_bc", bufs=1, name="den_bc")
                if no_den:
                    nc.vector.tensor_copy(den_bc[:], eacc[:])
                else:
                    nc.gpsimd.partition_all_reduce(den_bc[:], eacc[:], channels=128,
                                                   reduce_op=bass_isa.ReduceOp.add)
                if retr_fp8:
                    # fp8 exp can crush an entire row to 0 -> den=0 -> NaN;
                    # eps keeps inv finite (ret=0 for such rows)
                    nc.vector.tensor_scalar(den_bc[:], den_bc[:], 1e-6, None,
                                            op0=mybir.AluOpType.add)
                inv_bc = wp.tile([128, T], F32, tag="inv_bc", bufs=2, name="inv_bc")
                nc.vector.reciprocal(inv_bc[:], den_bc[:])

                if dbg and it == 0:
                    nc.sync.dma_start(dbg_expT[:],
                                      expT.rearrange("p a b -> p (a b)")[:])
                    nc.sync.dma_start(dbg_inv[:], inv_bc[:])

                # retrieved^T + h^T (in place over xT) + |h| max on Pool
                for eb in range(NE):
                    psr = psp.tile([128, T], F32, tag="ps_r", bufs=3, name="r_ps")
                    if retr_fp8:
                        pairs = [8, 10, 12, 14, 0, 2, 4, 6]
                        for i, mb in enumerate(pairs):
                            nc.tensor.matmul(
                                psr[:],
                                mv_sb[:, mb:mb + 2, eb * 128:(eb + 1) * 128],
                                expT[:, mb:mb + 2, :],
                                start=(i == 0), stop=(i == len(pairs) - 1),
                                perf_mode=mybir.MatmulPerfMode.DoubleRow)
                    else:
                        order = list(range(8, NM)) + list(range(8))
                        for i, mb in enumerate(order):
                            nc.tensor.matmul(
                                psr[:], mv_sb[:, mb, eb * 128:(eb + 1) * 128],
                                expT[:, mb, :],
                                start=(i == 0), stop=(i == NM - 1))
                    # normalize in place in PSUM, then accumulate into hT
                    nc.vector.tensor_tensor(psr[:], psr[:], inv_bc[:],
                                            op=mybir.AluOpType.mult)
                    hsl = hT[:, eb, t0:t0 + T]
                    nc.vector.tensor_tensor(hsl, psr[:], hsl,
                                            op=mybir.AluOpType.add)
                    nc.vector.tensor_reduce(
                        hmax_buf[:, it * NE + eb:it * NE + eb + 1], hsl,
                        axis=mybir.AxisListType.X, op=mybir.AluOpType.max,
                        apply_absolute_value=True)

                # Wo prep slice rides under this tile (idle rings/engines)
                if it in wo_slices:
                    lo = wo_slices.get(it - 1, 0)
                    for w in wo_work[lo:wo_slices[it]]:
                        w()

                if it + 1 < NT:
                    qxT = qxT_next_buf

            if dbg:
                nc.sync.dma_start(dbg_hT[:], hT.rearrange("p a b -> p (a b)")[:])

            wo_transpose()

            # ---- global max|h| -> s_h, output scales ----
            nc.vector.tensor_reduce(hmax_red[:], hmax_buf[:],
                                    axis=mybir.AxisListType.X,
                                    op=mybir.AluOpType.max)
            part_reduce_scalar(hmax_red, sc["red1"], "max")
            allreduce_max(sc["red1"], sc["gmax_h"])
            nc.vector.tensor_scalar(sc["s_h"][0:1, 0:1], sc["gmax_h"][0:1, 0:1],
                                    1.0 / 127.0, None, op0=mybir.AluOpType.mult)
            nc.vector.reciprocal(sc["inv_sh"][0:1, 0:1], sc["s_h"][0:1, 0:1])
            nc.gpsimd.partition_broadcast(bc["inv_sh"][:], sc["inv_sh"][0:1, 0:1])
            nc.vector.tensor_tensor(sc["sco"][0:1, 0:1], sc["ws_o"][0:1, 0:1],
                                    sc["s_h"][0:1, 0:1], op=mybir.AluOpType.mult)
            nc.gpsimd.partition_broadcast(bc["sco"][:], sc["sco"][0:1, 0:1])
            if with_bias:
                nc.vector.reciprocal(sc["inv_sco"][0:1, 0:1], sc["sco"][0:1, 0:1])
                nc.vector.tensor_scalar(bo_sc[0:1, :], bo_sb[0:1, :],
                                        sc["inv_sco"][0:1, 0:1], None,
                                        op0=mybir.AluOpType.mult)

            # ======================= PHASE C =======================
            def quantize_h(it):
                # in-place rne(hT/s_h); magic-round alternates ACT/DVE to
                # halve the serial latency at the B->C transition
                for eb in range(NE):
                    hsl = hT[:, eb, it * T:(it + 1) * T]
                    qt = wp.tile([128, T], F32, tag="inv_bc", bufs=2, name="qh")
                    if eb % 2 == 0:
                        nc.scalar.activation(
                            qt[:], hsl, mybir.ActivationFunctionType.Identity,
                            bias=magic_bc[:, 0:1], scale=bc["inv_sh"][:, 0:1])
                    else:
                        nc.vector.tensor_scalar(
                            qt[:], hsl, bc["inv_sh"][:, 0:1], magic_bc[:, 0:1],
                            op0=mybir.AluOpType.mult, op1=mybir.AluOpType.add)
                    nc.vector.tensor_scalar(hsl, qt[:], MAGIC, None,
                                            op0=mybir.AluOpType.subtract)

            quantize_h(0)
            for it in range(NT):
                t0 = it * T
                if it + 1 < NT:
                    quantize_h(it + 1)
                for tsub in range(TS):
                    tc0 = t0 + tsub * 128
                    # both output halves accumulate together so consecutive
                    # matmuls share the same lhsT (Ldweights dedup)
                    opss = [psp.tile([128, 512], F32, tag="ps_a", bufs=3,
                                     name="o_ps") for _ in range(NOH)]
                    for eb in range(NE):
                        for oh in range(NOH):
                            of = slice(oh * 512, (oh + 1) * 512)
                            nc.tensor.matmul(
                                opss[oh][:], hT[:, eb, tc0:tc0 + 128],
                                qWo[:, eb, of],
                                start=(eb == 0),
                                stop=(not with_bias and eb == NE - 1))
                    for oh in range(NOH):
                        of = slice(oh * 512, (oh + 1) * 512)
                        if with_bias:
                            nc.tensor.matmul(opss[oh][:], ones_row[0:1, :],
                                             bo_sc[0:1, of],
                                             start=False, stop=True)
                        osb = wp.tile([128, 512], F32, tag="eacc", bufs=2,
                                      name="osb")
                        nc.scalar.activation(
                            osb[:], opss[oh][:],
                            mybir.ActivationFunctionType.Copy,
                            bias=0.0, scale=bc["sco"][:, 0:1])
                        nc.scalar.dma_start(out_ext[tc0:tc0 + 128, of], osb[:])

    nc.compile()
    if dedup_ldw:
        _dedup_ldweights(nc)
    return nc


def _dedup_ldweights(nc):
    """Remove an InstLdweights identical to the previous one when only
    matmuls/event-semaphores sit between (weights already resident).
    Its waits move onto the following matmul."""
    removed = 0
    for blk in nc.m.functions[0].blocks:
        insts = list(blk.instructions)
        last_key = None
        drop = []          # indices to delete
        carry = None       # waits carried from a dropped LDW
        for i, inst in enumerate(insts):
            tn = type(inst).__name__
            if tn == "InstLdweights":
                pap = inst.ins[0]
                key = (getattr(pap, "memref", None), getattr(pap, "offset", None),
                       str(getattr(pap, "ap", None)), str(getattr(pap, "dtype", None)))
                si = inst.sync_info
                ups = list(si.on_update) if si else []
                if key == last_key and not ups:
                    drop.append(i)
                    if si and len(list(si.on_wait)):
                        carry = (list(si.on_wait), i)
                else:
                    last_key = key
            elif tn == "InstMatmult":
                if carry is not None:
                    w, _ = carry
                    si = inst.sync_info
                    if si is None:
                        inst.sync_info = mybir.SyncInfo(on_wait=w, on_update=[])
                    else:
                        si.on_wait = list(si.on_wait) + w
                    carry = None
            elif tn == "InstEventSemaphore":
                pass
            else:
                last_key = None
                if carry is not None:   # shouldn't happen; keep the LDW then
                    drop.remove(carry[1])
                    carry = None
        for i in reversed(drop):
            del blk.instructions[i]
        removed += len(drop)
    return removed


# ----------------------------------------------------------------------------
_CACHE = {}


def _get_nc(key="full", **kw):
    if key not in _CACHE:
        _CACHE[key] = build_nc(**kw)
    return _CACHE[key]


def _make_in_maps(x, memory_keys, memory_values, Wk, bk, Wo, bo):
    x = np.ascontiguousarray(x, dtype=np.float32)
    shared = {
        "memory_keys": np.ascontiguousarray(memory_keys, dtype=np.float32),
        "memory_values": np.ascontiguousarray(memory_values, dtype=np.float32),
        "Wk": np.ascontiguousarray(Wk, dtype=np.float32),
        "bk": np.ascontiguousarray(bk, dtype=np.float32),
        "Wo": np.ascontiguousarray(Wo, dtype=np.float32),
        "bo": np.ascontiguousarray(bo, dtype=np.float32),
    }
    return [dict(shared, x=x[i]) for i in range(x.shape[0])]


def kernel(x, memory_keys, memory_values, Wk, bk, Wv=None, bv=None, Wo=None, bo=None):
    wb = bool(np.any(np.asarray(bo)))
    wk_b = bool(np.any(np.asarray(bk)))
    nc = _get_nc(("full", wb, wk_b), with_bias=wb, with_bk=wk_b)
    in_maps = _make_in_maps(x, memory_keys, memory_values, Wk, bk, Wo, bo)
    res = run_bass_kernel_spmd(nc, in_maps, core_ids=list(range(N_CORES)))
    out = np.stack([res.results[i]["out"] for i in range(N_CORES)], axis=0)
    return out.astype(np.float32)


# ------------------------- benchmarking helper ------------------------------
def bench(inputs, iters=5, nc=None):
    """Time on-device execution with device-resident inputs."""
    import jax
    from jax.sharding import Mesh, PartitionSpec, NamedSharding
    from jax.experimental.shard_map import shard_map
    from concourse import bass2jax as b2j

    if nc is None:
        wb = bool(np.any(np.asarray(inputs["bo"])))
        wk_b = bool(np.any(np.asarray(inputs["bk"])))
        nc = _get_nc(("full", wb, wk_b), with_bias=wb, with_bk=wk_b)
    in_maps = _make_in_maps(inputs["x"], inputs["memory_keys"],
                            inputs["memory_values"], inputs["Wk"], inputs["bk"],
                            inputs["Wo"], inputs["bo"])
    b2j.install_neuronx_cc_hook()

    partition_name = nc.partition_id_tensor.name if nc.partition_id_tensor else None
    in_names, out_names, out_avals, zero_outs = [], [], [], []
    for alloc in nc.m.functions[0].allocations:
        if not isinstance(alloc, mybir.MemoryLocationSet):
            continue
        name = alloc.memorylocations[0].name
        if alloc.kind == "ExternalInput":
            if name != partition_name:
                in_names.append(name)
        elif alloc.kind == "ExternalOutput":
            out_names.append(name)
            shape = tuple(alloc.tensor_shape)
            dtype = mybir.dt.np(alloc.dtype)
            out_avals.append(jax.core.ShapedArray(shape, dtype))
            zero_outs.append(np.zeros(shape, dtype))
    n_params = len(in_names)
    n_outs = len(out_avals)
    in_names = in_names + out_names
    if partition_name is not None:
        in_names.append(partition_name)

    def _body(*args):
        operands = list(args)
        if partition_name is not None:
            operands.append(b2j.partition_id_tensor())
        outs = b2j._bass_exec_p.bind(
            *operands, out_avals=tuple(out_avals), in_names=tuple(in_names),
            out_names=tuple(out_names), lowering_input_output_aliases=(),
            sim_require_finite=True, sim_require_nnan=True, nc=nc)
        return tuple(outs)

    n_cores = len(in_maps)
    devices = jax.devices()[:n_cores]
    mesh = Mesh(np.asarray(devices), ("core",))
    in_specs = (PartitionSpec("core"),) * (n_params + n_outs)
    out_specs = (PartitionSpec("core"),) * len(out_names)
    donate = tuple(range(n_params, n_params + n_outs))
    sharded = jax.jit(
        shard_map(_body, mesh=mesh, in_specs=in_specs, out_specs=out_specs,
                  check_rep=False),
        donate_argnums=donate, keep_unused=True)

    per_core = [[np.asarray(m[nm]) for nm in in_names[:n_params]] for m in in_maps]
    concat_in = [np.concatenate([per_core[c][i] for c in range(n_cores)], axis=0)
                 for i in range(n_params)]
    sh = NamedSharding(mesh, PartitionSpec("core"))
    dev_in = [jax.device_put(a, sh) for a in concat_in]
    for a in dev_in:
        a.block_until_ready()

    times = []
    out_arrs = None
    for i in range(iters + 1):
        dev_zeros = [jax.device_put(
            np.zeros((n_cores * z.shape[0], *z.shape[1:]), z.dtype), sh)
            for z in zero_outs]
        for a in dev_zeros:
            a.block_until_ready()
        t0 = time.perf_counter()
        out_arrs = sharded(*dev_in, *dev_zeros)
        for o in out_arrs:
            o.block_until_ready()
        t1 = time.perf_counter()
        if i > 0:
            times.append(t1 - t0)
    oi = out_names.index("out")
    oshape = out_avals[oi].shape
    out = np.asarray(out_arrs[oi]).reshape(n_cores, *oshape)
    return times, out

